# revision 18
# baseline (speedup 1.0000x reference)
"""KimiLinear KDA decode step — Trainium2 Bass kernel (8 NeuronCores).

Problem: B=128 decode batch, HK=HV=32 heads, D=128 head dim, K=4 causal conv.
  1. per-channel causal conv1d update + silu over mixed_qkv (12288 channels)
  2. split q/k/v, l2norm(q)*D^-0.5, l2norm(k)
  3. fused KDA gate g = -exp(A_log)*softplus(forget_gate + dt_bias), b=sigmoid(beta)
  4. gated delta-rule readout:
       o = mg @ S + cc*v   with  cc = (q.k)*b,  mg = q*eg - cc*k*eg
     (the updated state is never materialized: one mat-vec against S per
     (b,h) plus a rank-1 correction).

Sharding: data-parallel over batch — 16 batches per core; each core handles all
32 heads of its batch slice with zero cross-core communication.

The kernel is memory-bound on the ssm_state read. Key choices:
  - ssm_state ships as a single fp16 copy (2 B/elem), pre-transposed
    host-side to k-major [chunk][k][b][h][v] so each SBUF partition line is
    one contiguous 16 KB DRAM read (peak-rate DMA descriptors), streamed in
    8 triple-buffered ~2.1 MB chunks.
  - the whole front-end (conv, norms, gate) runs in fp16 on DVE/ACT (2x
    DVE perf mode), with conv weights / dt_bias / A_log kept unreplicated
    in SBUF and broadcast via stride-0 access patterns.
  - activation layout: [128 partitions = d, free = (b, sec, h)] so the
    conv is elementwise, q/k/v are matmul-ready on the contraction (d)
    partition axis, and per-(b,h) scalars broadcast with tiny ones-matmuls.

Per (b,h): one fp16 matmul, stationary = S[b,h] (128x128, FWL fast path),
moving = the folded query vector mg (1 column), output = one PSUM column.
All 512 outputs pack into a single PSUM bank, evacuated once at the end
with the cc*v correction fused into the copy.
"""

import numpy as np

import concourse.bass as bass
import concourse.bacc as bacc
import concourse.mybir as mybir
from concourse.tile import TileContext
from concourse.bass_utils import run_bass_kernel_spmd

F32 = mybir.dt.float32
F16 = mybir.dt.float16
AF = mybir.ActivationFunctionType
OP = mybir.AluOpType

NCORES = 8
B, HK, HV, D, CK = 128, 32, 32, 128, 4
SEC = 3                      # q | k | v channel sections of 32 heads each
SH = SEC * HV                # 96
BC = B // NCORES             # batches per core = 16
NHB = HV * BC                # per-(b,h) columns = 512
QKV = (2 * HK + HV) * D      # 12288
NCHUNK = 8                   # ssm stream chunks per core
CB = BC // NCHUNK            # batches per chunk = 2

_CACHE = {}


def _build_nc():
    # Bacc (not raw Bass): its compile() splits multi-sem waits into event
    # semaphores — TRN2 instructions carry at most one wait.
    nc = bacc.Bacc("TRN2", target_bir_lowering=False, debug=False)
    S3 = SEC * NHB  # 1536, cols ordered (b, sec, h)
    # win = [conv_state taps j=0..2 | mixed_qkv] in layout (j, b, sec, h)
    win = nc.declare_dram_parameter("win", [D, CK * S3], F16, isOutput=False)
    w16 = nc.declare_dram_parameter("w16", [D, CK * SH], F16, isOutput=False)
    # aux = [forget_gate (b h) | beta (b h) | dt_bias (h) | -exp(A_log) (h)]
    aux = nc.declare_dram_parameter("aux", [D, 2 * NHB + 2 * HV], F16,
                                    isOutput=False)
    # ssm as fp16, k-major: [chunk][k][b_local][h][v]; each (chunk, k) row is
    # a contiguous 16 KB DRAM read feeding one SBUF partition.
    s16 = nc.declare_dram_parameter("s16", [NCHUNK, D, CB * HV * D], F16,
                                    isOutput=False)
    o_out = nc.declare_dram_parameter("o_out", [D, NHB], F32, isOutput=True)

    with TileContext(nc) as tc:
        with (
            tc.tile_pool(name="const", bufs=1) as const,
            tc.tile_pool(name="work", bufs=1) as work,
            tc.tile_pool(name="spool", bufs=7) as spool,
            tc.tile_pool(name="psr", bufs=1, space="PSUM") as psr,
            tc.tile_pool(name="psb", bufs=1, space="PSUM") as psb,
            tc.tile_pool(name="pso", bufs=1, space="PSUM") as pso,
        ):
            # ---- input staging --------------------------------------------
            t_win = const.tile([D, CK * S3], F16)
            nc.sync.dma_start(t_win[:], win[:])
            t_w = const.tile([D, CK * SH], F16)
            nc.sync.dma_start(t_w[:], w16[:])
            t_aux = const.tile([D, 2 * NHB + 2 * HV], F16)
            nc.scalar.dma_start(t_aux[:], aux[:])

            ones_c = const.tile([D, 1], F16)
            nc.vector.memset(ones_c[:], 1.0)
            ones_r = const.tile([1, D], F16)
            nc.vector.memset(ones_r[:], 1.0)
            halfr = const.tile([1, NHB], F16)
            nc.vector.memset(halfr[:], 0.5)
            # register the q-half rsqrt bias (-0.5*ln(D)) as a const AP so
            # scalar.activation can use it as a bias operand
            import math
            bias_q = -0.5 * math.log(D)
            cbias = const.tile([D, 1], F32)
            nc.vector.memset(cbias[:], bias_q)
            nc.const_aps.aps[(F32, bias_q)] = cbias[:]

            fg_v = t_aux[:, 0:NHB].rearrange("p (b h) -> p b h", b=BC)
            beta_row = t_aux[0:1, NHB:2 * NHB]
            dtb_v = t_aux[:, 2 * NHB:2 * NHB + HV] \
                .unsqueeze(1).broadcast_to([D, BC, HV])
            nega_v = t_aux[:, 2 * NHB + HV:2 * NHB + 2 * HV] \
                .unsqueeze(1).broadcast_to([D, BC, HV])

            # ACT op 1 (silu table): bsig row via tanh — sigmoid(x) =
            # 0.5*tanh(x/2) + 0.5, finished as brow on DVE.
            bsig_t = work.tile([1, NHB], F16)
            nc.scalar.activation(bsig_t[:], beta_row, AF.Tanh, scale=0.5)

            # ---- causal conv1d single-step + silu -------------------------
            # prod[d, j, b, (sec h)] = win * w (w broadcast over b)
            g1 = work.tile([D, NHB], F16)
            g1_v = g1[:].rearrange("p (b h) -> p b h", b=BC)
            nc.vector.tensor_tensor(g1_v, fg_v, dtb_v, OP.add)
            prod = work.tile([D, CK * S3], F16)
            win_v = t_win[:].rearrange("p (j b sh) -> p j b sh", j=CK, b=BC)
            prod_v = prod[:].rearrange("p (j b sh) -> p j b sh", j=CK, b=BC)
            wb = t_w[:].rearrange("p (j sh) -> p j sh", j=CK) \
                .unsqueeze(2).broadcast_to([D, CK, BC, SH])
            nc.vector.tensor_tensor(prod_v, win_v, wb, OP.mult)
            t01 = work.tile([D, 2 * S3], F16)
            nc.vector.tensor_tensor(t01[:], prod[:, 0:2 * S3],
                                    prod[:, 2 * S3:4 * S3], OP.add)
            accc = work.tile([D, S3], F16)
            nc.vector.tensor_tensor(accc[:], t01[:, 0:S3], t01[:, S3:2 * S3],
                                    OP.add)
            x = work.tile([D, S3], F16)
            nc.scalar.activation(x[:], accc[:], AF.Silu)  # silu table (loaded)
            xv = x[:].rearrange("p (b s h) -> p b s h", b=BC, s=SEC)
            x_t = x[:].rearrange("p (b s h) -> p s b h", b=BC, s=SEC)

            # DVE ops that only need g1 / bsig_t — keep DVE busy early
            gr = work.tile([D, NHB], F16)
            nc.vector.tensor_scalar_max(gr[:], g1[:], 0.0)
            brow = work.tile([1, NHB], F16)
            nc.vector.scalar_tensor_tensor(
                brow[:], bsig_t[:], 0.5, halfr[:], OP.mult, OP.add)

            # ---- l2 norms + raw q.k (partition reduce via ones-matmul) ----
            sq = work.tile([D, 2 * NHB], F16)   # cols (t, b, h), t = q|k
            sq_v = sq[:].rearrange("p (t b h) -> p t b h", t=2, b=BC)
            nc.vector.tensor_tensor(sq_v, x_t[:, 0:2], x_t[:, 0:2], OP.mult)
            sq2 = work.tile([D, NHB], F16)      # q_raw * k_raw
            nc.vector.tensor_tensor(sq2[:], x_t[:, 0], x_t[:, 1], OP.mult)
            nrow = psr.tile([1, 2 * NHB], F32)
            nc.tensor.matmul(nrow[:, 0:NHB], ones_c[:], sq[:, 0:NHB],
                             start=True, stop=True)
            nc.tensor.matmul(nrow[:, NHB:2 * NHB], ones_c[:], sq[:, NHB:2 * NHB],
                             start=True, stop=True)
            qkrow = psr.tile([1, NHB], F32)
            nc.tensor.matmul(qkrow[:], ones_c[:], sq2[:], start=True, stop=True)
            # eps-add early so the rsqrt -> broadcast -> mg chain starts ASAP
            neps = work.tile([1, 2 * NHB], F32)
            nc.vector.tensor_scalar_add(neps[:], nrow[:], 1e-6)

            # ---- KDA gate: eg = exp(-exp(A_log)*softplus(fg+dt_bias)) -----
            # no softplus ACT table: softplus(x) = relu(x) + ln(1+exp(-|x|));
            # abs/exp/ln/copy all live in one ACT table.
            ga = work.tile([D, NHB], F16)
            nc.scalar.activation(ga[:], g1[:], AF.Abs)
            nc.scalar.activation(ga[:], ga[:], AF.Exp, scale=-1.0)
            nc.scalar.activation(ga[:], ga[:], AF.Ln, bias=1.0)
            sp = work.tile([D, NHB], F16)
            nc.vector.tensor_tensor(sp[:], gr[:], ga[:], OP.add)
            gs = work.tile([D, NHB], F16)
            gs_v = gs[:].rearrange("p (b h) -> p b h", b=BC)
            sp_v = sp[:].rearrange("p (b h) -> p b h", b=BC)
            nc.vector.tensor_tensor(gs_v, sp_v, nega_v, OP.mult)

            # rsqrt of norms via exp(-0.5*ln(x)) — Rsqrt/Reciprocal ACT
            # tables are unavailable, DVE reciprocal is 6.5us.
            lnr = work.tile([1, 2 * NHB], F32)
            nc.scalar.activation(lnr[:], neps[:], AF.Ln)
            eg = work.tile([D, NHB], F16)
            nc.scalar.activation(eg[:], gs[:], AF.Exp)
            # rows3 = [rsq_q * D^-0.5 | cc*rsq_k | cc] broadcast targets
            rows3 = work.tile([1, 3 * NHB], F16)
            srow_q = rows3[:, 0:NHB]
            nc.scalar.activation(srow_q, lnr[:, 0:NHB], AF.Exp, scale=-0.5,
                                 bias=bias_q)
            srow_k = work.tile([1, NHB], F16)
            nc.scalar.activation(srow_k[:], lnr[:, NHB:2 * NHB], AF.Exp,
                                 scale=-0.5)

            # xe = x_qk * eg (eg broadcast over t) — independent of norms
            xe = work.tile([D, 2 * NHB], F16)
            xe_v = xe[:].rearrange("p (t f) -> p t f", t=2)
            eg_b = eg[:].unsqueeze(1).broadcast_to([D, 2, NHB])
            nc.vector.tensor_tensor(xe_v, x_t[:, 0:2], eg_b, OP.mult)

            # row-space fold: cc = qk_raw*rsq_q*rsq_k*sigmoid(beta)
            n1 = work.tile([1, NHB], F16)
            nc.vector.tensor_tensor(n1[:], qkrow[:], brow[:], OP.mult)
            n2 = work.tile([1, NHB], F16)
            nc.vector.tensor_tensor(n2[:], n1[:], srow_q, OP.mult)
            ccrow = rows3[:, 2 * NHB:3 * NHB]
            nc.vector.tensor_tensor(ccrow, n2[:], srow_k[:], OP.mult)
            crow = rows3[:, NHB:2 * NHB]
            nc.vector.tensor_tensor(crow, ccrow, srow_k[:], OP.mult)

            # broadcast all three rows along partitions in one PSUM tile
            rb3 = psb.tile([D, 3 * NHB], F32)
            nc.tensor.matmul(rb3[:, 0:NHB], ones_r[:], rows3[:, 0:NHB],
                             start=True, stop=True)
            nc.tensor.matmul(rb3[:, NHB:2 * NHB], ones_r[:],
                             rows3[:, NHB:2 * NHB], start=True, stop=True)
            nc.tensor.matmul(rb3[:, 2 * NHB:3 * NHB], ones_r[:],
                             rows3[:, 2 * NHB:3 * NHB], start=True, stop=True)

            # mg = q_raw*eg*rsq_q*D^-0.5 - k_raw*eg*(cc*rsq_k)
            qkgc = work.tile([D, 2 * NHB], F16)
            nc.vector.tensor_tensor(qkgc[:], xe[:], rb3[:, 0:2 * NHB], OP.mult)
            mg = work.tile([D, NHB], F16)
            nc.vector.tensor_tensor(mg[:], qkgc[:, 0:NHB],
                                    qkgc[:, NHB:2 * NHB], OP.subtract)
            cv = work.tile([D, NHB], F32)
            cv_v = cv[:].rearrange("p (b h) -> p b h", b=BC)
            ccb_v = rb3[:, 2 * NHB:3 * NHB].rearrange("p (b h) -> p b h", b=BC)
            nc.vector.tensor_tensor(cv_v, xv[:, :, 2, :], ccb_v, OP.mult)

            # ---- main loop: stream S chunks, one fp16 mat-vec per (b,h) ---
            # PSUM: output columns split across two banks (chunks 0-3 and
            # 4-7) so the first half evacuates + stores to HBM while the
            # second half is still accumulating. Columns ordered (b, h).
            HB = NHB // 2
            o_psA = pso.tile([D, NHB], F32)
            o_psB = pso.tile([D, NHB], F32)
            o_t = work.tile([D, NHB], F32)
            for c in range(NCHUNK):
                St = spool.tile([D, CB * HV, D], F16, name="St", tag="St")
                nc.sync.dma_start(St[:], s16[c])
                ps = o_psA if c < NCHUNK // 2 else o_psB
                off = 0 if c < NCHUNK // 2 else HB
                for bl in range(CB):
                    for h in range(HV):
                        col = (c * CB + bl) * HV + h
                        nc.tensor.matmul(
                            ps[:, col - off:col - off + 1],
                            St[:, bl * HV + h, :],
                            mg[:, col:col + 1], start=True, stop=True)
                if c == NCHUNK // 2 - 1:
                    # evacuate first half while the second half matmuls run
                    nc.vector.scalar_tensor_tensor(
                        o_t[:, 0:HB], o_psA[:, 0:HB], 1.0, cv[:, 0:HB],
                        OP.mult, OP.add)
                    nc.scalar.dma_start(o_out[:, 0:HB], o_t[:, 0:HB])

            nc.vector.scalar_tensor_tensor(
                o_t[:, HB:NHB], o_psB[:, 0:HB], 1.0, cv[:, HB:NHB],
                OP.mult, OP.add)
            nc.scalar.dma_start(o_out[:, HB:NHB], o_t[:, HB:NHB])

    nc.compile()
    return nc


def _prep_bsh(a):
    """[bc, sec*32*128] activation slice -> [128 d, (b, sec, h)] layout."""
    bc = a.shape[0]
    return a.reshape(bc, SEC, HV, D).transpose(3, 0, 1, 2).reshape(D, bc * SH)


def _prep_inputs(mixed_qkv, forget_gate, beta, conv_state, conv_weights,
                 ssm_state, A_log, dt_bias):
    mixed_qkv = np.asarray(mixed_qkv, np.float32)
    forget_gate = np.asarray(forget_gate, np.float32)
    beta = np.asarray(beta, np.float32)
    conv_state = np.asarray(conv_state, np.float32)
    conv_weights = np.asarray(conv_weights, np.float32)
    ssm_state = np.asarray(ssm_state, np.float32)
    A_log = np.asarray(A_log, np.float32)
    dt_bias = np.asarray(dt_bias, np.float32)

    # shared (weight) tensors
    w16 = np.ascontiguousarray(
        conv_weights.reshape(SEC, HV, D, CK).transpose(2, 3, 0, 1)
        .reshape(D, CK * SH)).astype(np.float16)
    dtb = dt_bias.reshape(HV, D).T                      # [D, HV]
    nega = np.broadcast_to((-np.exp(A_log))[None, :], (D, HV))

    in_maps = []
    for c in range(NCORES):
        cs = slice(c * BC, (c + 1) * BC)
        cstc = conv_state[cs]  # [BC, QKV, 3]
        win = np.concatenate(
            [_prep_bsh(cstc[:, :, j]) for j in range(CK - 1)]
            + [_prep_bsh(mixed_qkv[cs])], axis=1).astype(np.float16)
        fgp = forget_gate[cs].reshape(BC, HV, D).transpose(2, 0, 1) \
            .reshape(D, NHB)
        betar = np.broadcast_to(beta[cs].reshape(1, NHB), (D, NHB))
        auxc = np.concatenate([fgp, betar, dtb, nega], axis=1) \
            .astype(np.float16)
        # k-major fp16 ssm: [chunk][k][b_local][h][v]
        s16 = np.ascontiguousarray(
            ssm_state[cs].reshape(NCHUNK, CB, HV, D, D)
            .transpose(0, 3, 1, 2, 4)
            .reshape(NCHUNK, D, CB * HV * D).astype(np.float16))
        in_maps.append({
            "win": np.ascontiguousarray(win),
            "w16": w16,
            "aux": np.ascontiguousarray(auxc),
            "s16": s16,
        })
    return in_maps


def run(trace=False, **inputs):
    if "nc" not in _CACHE:
        _CACHE["nc"] = _build_nc()
    nc = _CACHE["nc"]
    in_maps = _prep_inputs(**inputs)
    res = run_bass_kernel_spmd(nc, in_maps, list(range(NCORES)), trace=trace)
    outs = []
    for c in range(NCORES):
        oc = np.asarray(res.results[c]["o_out"])  # [128, 512] cols (b, h)
        outs.append(oc.reshape(D, BC, HV).transpose(1, 2, 0))  # [BC, HV, D]
    return np.concatenate(outs, axis=0), res


def kernel(**inputs) -> np.ndarray:
    out, _ = run(trace=False, **inputs)
    return out
